# revision 46
# baseline (speedup 1.0000x reference)
"""Bispectrum kernel for Trainium2 (8 NeuronCores, batch-parallel).

For each batch row x (length N=1024):
    y  = FFT(x)
    Bx[i,j] = y_i * conj(y_j) * y_{(j-i) % N}
    out[b]  = stack([Bx.real, Bx.imag])            # [2, N, N] fp32

Device strategy per core (4 samples):
  * host precomputes y (tiny FFT, 0.001% of FLOPs) and ships:
      - bf16 hi/lo split tables so PE computes A = y_i*conj(y_j) outer
        products as K=6 bf16 matmuls at full speed (~1e-5 rel err)
      - a "superplane" per sample/component: SP[p,t] = y2[640+t-p], the
        union of the 4 circulant planes C[i,j] = y_{(j-i)%N} needed by
        row-blocks 0..3 (DMA-legal, 4.3x smaller than per-block planes)
  * DVE does the 4 elementwise multiplies (A.C complex product),
    GpSimd the 2 add/subs -> Bx rows 0..511 (+row 512 epilogue)
  * conjugate symmetry Bx[N-i, N-j] = conj(Bx[i,j]) produces rows
    513..1023: PE multiplies by an anti-identity (exact fp32) to reverse
    partitions, ScalarE evacuates PSUM with a reversed-column read
    (negated for imag), stores are plain ascending DMAs.
"""

import os
import numpy as np

B = 32
N = 1024
NCORES = 8
BS = B // NCORES          # samples per core
NBLK = 4                  # computed 128-row blocks per sample (rows 0..511)
W = 1408                  # superplane width

_cache = {}

_SIM = os.environ.get("KERNEL_SIM", "0") == "1"

last_exec_time_ns = None
last_trace = None


def _emit(nc, tc, ctx, bassmod, aps):
    import concourse.bass as bass
    import concourse.mybir as mybir

    f32 = mybir.dt.float32
    bf16 = mybir.dt.bfloat16
    P = 128

    ta1d, ta2d, trd, t1d, jmatd, spd, srcd = aps

    const = ctx.enter_context(tc.tile_pool(name="const", bufs=1))
    gpool = ctx.enter_context(tc.tile_pool(name="gather", bufs=BS))
    mpool = ctx.enter_context(tc.tile_pool(name="mtiles", bufs=2))
    bxpool = ctx.enter_context(tc.tile_pool(name="bx", bufs=2))
    fpool = ctx.enter_context(tc.tile_pool(name="flip", bufs=3))
    pa = ctx.enter_context(tc.tile_pool(name="pa", bufs=2, space="PSUM"))
    pj = ctx.enter_context(tc.tile_pool(name="pj", bufs=2, space="PSUM"))
    spool = ctx.enter_context(tc.tile_pool(name="small", bufs=1))

    REV32 = list(range(31, -1, -1))   # reverse within each 32-lane quadrant

    # constant tables
    ta1sb = const.tile([6, BS * N], bf16)
    nc.sync.dma_start(ta1sb.rearrange("p (s n) -> p s n", s=BS),
                      ta1d.transpose([1, 0, 2]))
    ta2sb = const.tile([6, BS * N], bf16)
    nc.sync.dma_start(ta2sb.rearrange("p (s n) -> p s n", s=BS),
                      ta2d.transpose([1, 0, 2]))
    trsb = const.tile([6, BS * N], bf16)
    nc.sync.dma_start(trsb.rearrange("p (s n) -> p s n", s=BS),
                      trd.transpose([1, 0, 2]))
    jmat = const.tile([P, P], f32)
    nc.sync.dma_start(jmat[:, :], jmatd[:, :])

    NN = N * N
    src_h = srcd.tensor

    def out_ap(s, row0, nrows):
        # [rows, comps, cols] walk of src[s, :, row0:row0+nrows, :]
        return bassmod.AP(src_h, s * 2 * NN + row0 * N,
                          [[N, nrows], [NN, 2], [1, N]])

    # epilogue inputs load first (tiny) so its DVE/ACT work fills the
    # startup gap while the first big gather is in flight
    yrALL = spool.tile([BS, N], f32)
    nc.sync.dma_start(yrALL[:, :], t1d[:, 0, :])
    yiALL = spool.tile([BS, N], f32)
    nc.sync.dma_start(yiALL[:, :], t1d[:, 1, :])

    # all superplane gathers issued up front so no store blocks them.
    # sample 0 is split per component so the first multiply only waits for
    # the real half (subtile deps let DVE start ~3us earlier).
    SGs = []
    for s in range(BS):
        # fused circulant superplane gather: SG[p, c*W + t] = y2c[639 + t - p]
        SG = gpool.tile([P, 2 * W], f32, tag="SG")
        if s == 0:
            nc.sync.dma_start(SG[:, 0:W], spd[s][:, 0, :])
            nc.sync.dma_start(SG[:, W:2 * W], spd[s][:, 1, :])
        else:
            nc.sync.dma_start(SG.rearrange("p (c t) -> p c t", c=2), spd[s])
        SGs.append(SG)

    # ---- row 0 (self-paired under the flip), batched across samples ----
    # Bx[0, j] = y_0 * conj(y_j) * y_j ; C row for i=0 is just y itself.

    yr0 = yrALL[:, 0:1]
    yi0 = yiALL[:, 0:1]

    tr1 = spool.tile([BS, N], f32, tag="tr1")
    nc.scalar.mul(tr1[:, :], yrALL[:, :], yr0)
    tr2 = spool.tile([BS, N], f32, tag="tr2")
    nc.scalar.mul(tr2[:, :], yiALL[:, :], yi0)
    A0r = spool.tile([BS, N], f32)
    nc.vector.tensor_add(A0r[:, :], tr1[:, :], tr2[:, :])

    ti1 = spool.tile([BS, N], f32, tag="ti1")
    nc.scalar.mul(ti1[:, :], yrALL[:, :], yi0)
    ti2 = spool.tile([BS, N], f32, tag="ti2")
    nc.scalar.mul(ti2[:, :], yiALL[:, :], yr0)
    A0i = spool.tile([BS, N], f32)
    nc.vector.tensor_sub(A0i[:, :], ti1[:, :], ti2[:, :])

    mm1 = spool.tile([BS, N], f32, tag="tr1")
    nc.vector.tensor_mul(mm1[:, :], A0r[:, :], yrALL[:, :])
    mm2 = spool.tile([BS, N], f32, tag="tr2")
    nc.vector.tensor_mul(mm2[:, :], A0i[:, :], yiALL[:, :])
    mm3 = spool.tile([BS, N], f32, tag="ti1")
    nc.vector.tensor_mul(mm3[:, :], A0r[:, :], yiALL[:, :])
    mm4 = spool.tile([BS, N], f32, tag="ti2")
    nc.vector.tensor_mul(mm4[:, :], A0i[:, :], yrALL[:, :])

    R0 = spool.tile([BS, 2 * N], f32)
    nc.gpsimd.tensor_sub(R0[:, 0:N], mm1[:, :], mm2[:, :])
    nc.gpsimd.tensor_add(R0[:, N:2 * N], mm3[:, :], mm4[:, :])

    r0_ap = bassmod.AP(src_h, 0, [[2 * NN, BS], [NN, 2], [1, N]])
    nc.sync.dma_start(r0_ap, R0.rearrange("p (c n) -> p c n", c=2))

    for s in range(BS):
        SG = SGs[s]
        for p0 in range(NBLK):
            # block rows: 1 + 128*p0 .. 128 + 128*p0  (rows 1..512)
            r_base = 1 + 128 * p0
            Ar = pa.tile([P, N], f32, tag="Ar")
            Ai = pa.tile([P, N], f32, tag="Ai", bufs=1)
            lhs1 = ta1sb[:, s * N + r_base: s * N + r_base + 128]
            lhs2 = ta2sb[:, s * N + r_base: s * N + r_base + 128]
            for h in range(2):
                rhs = trsb[:, s * N + 512 * h: s * N + 512 * h + 512]
                nc.tensor.matmul(Ar[:, 512 * h: 512 * h + 512], lhs1, rhs,
                                 start=True, stop=True)
                nc.tensor.matmul(Ai[:, 512 * h: 512 * h + 512], lhs2, rhs,
                                 start=True, stop=True)

            c0 = 384 - 128 * p0
            Crs = SG[:, c0:c0 + N]
            Cis = SG[:, W + c0:W + c0 + N]

            # order: both Ar readers first, then Ai readers (frees Ar early)
            m1 = mpool.tile([P, N], f32, tag="m1")
            nc.vector.tensor_mul(m1[:, :], Ar[:, :], Crs)
            m3 = mpool.tile([P, N], f32, tag="m3")
            nc.vector.tensor_mul(m3[:, :], Ar[:, :], Cis)
            m2 = mpool.tile([P, N], f32, tag="m2")
            nc.vector.tensor_mul(m2[:, :], Ai[:, :], Cis)
            m4 = mpool.tile([P, N], f32, tag="m4")
            nc.vector.tensor_mul(m4[:, :], Ai[:, :], Crs)

            # fused [real | imag] plane
            Bx = bxpool.tile([P, 2 * N], f32, tag="Bx")
            nc.gpsimd.tensor_sub(Bx[:, 0:N], m1[:, :], m2[:, :])
            nc.gpsimd.tensor_add(Bx[:, N:2 * N], m3[:, :], m4[:, :])

            # direct store
            nc.sync.dma_start(out_ap(s, r_base, 128),
                              Bx.rearrange("p (c n) -> p c n", c=2))

            # conjugate flip, split across engines:
            #  - real plane: full row reversal on PE (anti-identity, exact
            #    fp32), ACT evacuates PSUM with reversed-column reads
            #  - imag plane: ACT negates, DVE quadrant-reverse shuffle with
            #    reversed-column read; quadrant swap folds into the stores
            F = fpool.tile([P, 2 * N], f32, tag="F")
            RBh0 = pj.tile([P, 512], f32, tag="RBh")
            nc.tensor.matmul(RBh0[:, :], jmat[:, :], Bx[:, 0:512],
                             start=True, stop=True)
            RBh1 = pj.tile([P, 512], f32, tag="RBh")
            nc.tensor.matmul(RBh1[:, :], jmat[:, :], Bx[:, 512:N],
                             start=True, stop=True)
            nc.scalar.copy(F[:, 1:512], RBh1[:, 511:0:-1])
            nc.scalar.copy(F[:, 512:513], RBh1[:, 0:1])
            nc.scalar.copy(F[:, 513:N], RBh0[:, 511:0:-1])
            nc.scalar.copy(F[:, 0:1], RBh0[:, 0:1])

            Tn = fpool.tile([P, N + 1], f32, tag="Tn")
            nc.scalar.mul(Tn[:, 0:N], Bx[:, N:2 * N], -1.0)
            nc.scalar.mul(Tn[:, N:N + 1], Bx[:, N:N + 1], -1.0)
            nc.vector.stream_shuffle(F[:, N:2 * N], Tn[:, N:0:-1], REV32)

            # derived stores (imag was only quadrant-reversed on DVE; real
            # was fully row-reversed on PE -> different partition mappings)
            # real: F[p] -> row 896 - 128*p0 + p ; imag: F[32q+u] -> row
            # (992 - 128*p0 - 32q) + u. Store the two comps separately.
            nc.sync.dma_start(
                bassmod.AP(src_h, s * 2 * NN + (896 - 128 * p0) * N,
                           [[N, 128], [1, N]]),
                F[:, 0:N])
            for q in range(4):
                nc.sync.dma_start(
                    bassmod.AP(src_h,
                               s * 2 * NN + NN + (992 - 128 * p0 - 32 * q) * N,
                               [[N, 32], [1, N]]),
                    F[32 * q:32 * q + 32, N:2 * N])

def _build():
    if "nc" in _cache:
        return _cache["nc"]
    from contextlib import ExitStack

    import concourse.bass as bass
    import concourse.bacc as bacc
    import concourse.mybir as mybir
    import concourse.tile as tile

    f32 = mybir.dt.float32
    bf16 = mybir.dt.bfloat16
    nc = bacc.Bacc("TRN2", target_bir_lowering=False, debug=False,
                   num_devices=NCORES)
    ta1d = nc.dram_tensor("ta1", [BS, 6, N], bf16, kind="ExternalInput").ap()
    ta2d = nc.dram_tensor("ta2", [BS, 6, N], bf16, kind="ExternalInput").ap()
    trd = nc.dram_tensor("tr", [BS, 6, N], bf16, kind="ExternalInput").ap()
    t1d = nc.dram_tensor("t1", [BS, 2, N], f32, kind="ExternalInput").ap()
    jmatd = nc.dram_tensor("jmat", [128, 128], f32, kind="ExternalInput").ap()
    spd = nc.dram_tensor("sp", [BS, 128, 2, W], f32, kind="ExternalInput").ap()
    srcd = nc.dram_tensor("src", [BS, 2, N, N], f32, kind="ExternalOutput").ap()

    aps = (ta1d, ta2d, trd, t1d, jmatd, spd, srcd)

    with tile.TileContext(nc) as tc:
        with ExitStack() as ctx:
            _emit(nc, tc, ctx, bass, aps)

    nc.compile()

    _cache["nc"] = nc
    return nc


def _host_prep(target):
    import ml_dtypes
    bf16 = ml_dtypes.bfloat16

    x = np.asarray(target)[:, 0, :].astype(np.float64)
    y = np.fft.fft(x)
    yr = np.ascontiguousarray(y.real.astype(np.float32))
    yi = np.ascontiguousarray(y.imag.astype(np.float32))

    yrh = yr.astype(bf16).astype(np.float32)
    yrl = (yr - yrh).astype(bf16).astype(np.float32)
    yih = yi.astype(bf16).astype(np.float32)
    yil = (yi - yih).astype(bf16).astype(np.float32)

    ta1 = np.ascontiguousarray(
        np.stack([yrh, yrh, yrl, yih, yih, yil], axis=1).astype(bf16))
    ta2 = np.ascontiguousarray(
        np.stack([yih, yih, yil, -yrh, -yrh, -yrl], axis=1).astype(bf16))
    tr = np.ascontiguousarray(
        np.stack([yrh, yrl, yrh, yih, yil, yih], axis=1).astype(bf16))
    t1 = np.ascontiguousarray(np.stack([yr, yi], axis=1))

    y2r = np.concatenate([yr, yr], axis=1)
    y2i = np.concatenate([yi, yi], axis=1)

    # superplane SP[s, p, c, t] = y2c[s, 639 + t - p]  (partition-major)
    pidx = np.arange(128)[:, None]
    tidx = np.arange(W)[None, :]
    gidx = 639 + tidx - pidx          # [128, W]
    sp = np.ascontiguousarray(
        np.stack([y2r[:, gidx], y2i[:, gidx]], axis=2))

    jmat = np.zeros((128, 128), np.float32)
    jmat[np.arange(128), 127 - np.arange(128)] = 1.0

    return ta1, ta2, tr, t1, jmat, sp


def kernel(**inputs):
    global last_exec_time_ns, last_trace
    target = np.asarray(inputs["target"], dtype=np.float32)
    ta1, ta2, tr, t1, jmat, sp = _host_prep(target)

    nc = _build()

    in_maps = []
    for c in range(NCORES):
        sl = slice(c * BS, (c + 1) * BS)
        in_maps.append({
            "ta1": ta1[sl], "ta2": ta2[sl], "tr": tr[sl], "t1": t1[sl],
            "jmat": jmat, "sp": sp[sl],
        })

    if _SIM:
        from concourse.bass_interp import CoreSim
        outs = []
        for c in range(NCORES):
            sim = CoreSim(nc, trace=False)
            for k, v in in_maps[c].items():
                sim.tensor(k)[:] = v
            sim.simulate(check_with_hw=False)
            outs.append(np.array(sim.tensor("src")))
        source = np.concatenate(outs, axis=0)
    else:
        from concourse.bass_utils import run_bass_kernel_spmd
        res = None
        for attempt in range(3):
            try:
                res = run_bass_kernel_spmd(nc, in_maps,
                                           core_ids=list(range(NCORES)))
                break
            except Exception:
                if attempt == 2:
                    raise
                import time as _time
                _time.sleep(2.0)
        last_exec_time_ns = res.exec_time_ns
        last_trace = res.instructions_and_trace
        source = np.concatenate([r["src"] for r in res.results], axis=0)

    return source, target


# revision 47
# speedup vs baseline: 1.0235x; 1.0235x over previous
"""Bispectrum kernel for Trainium2 (8 NeuronCores, batch-parallel).

For each batch row x (length N=1024):
    y  = FFT(x)
    Bx[i,j] = y_i * conj(y_j) * y_{(j-i) % N}
    out[b]  = stack([Bx.real, Bx.imag])            # [2, N, N] fp32

Device strategy per core (4 samples):
  * host precomputes y (tiny FFT, 0.001% of FLOPs) and ships:
      - bf16 hi/lo split tables so PE computes A = y_i*conj(y_j) outer
        products as K=6 bf16 matmuls at full speed (~1e-5 rel err)
      - a "superplane" per sample/component: SP[p,t] = y2[640+t-p], the
        union of the 4 circulant planes C[i,j] = y_{(j-i)%N} needed by
        row-blocks 0..3 (DMA-legal, 4.3x smaller than per-block planes)
  * DVE does the 4 elementwise multiplies (A.C complex product),
    GpSimd the 2 add/subs -> Bx rows 0..511 (+row 512 epilogue)
  * conjugate symmetry Bx[N-i, N-j] = conj(Bx[i,j]) produces rows
    513..1023: PE multiplies by an anti-identity (exact fp32) to reverse
    partitions, ScalarE evacuates PSUM with a reversed-column read
    (negated for imag), stores are plain ascending DMAs.
"""

import os
import numpy as np

B = 32
N = 1024
NCORES = 8
BS = B // NCORES          # samples per core
NBLK = 4                  # computed 128-row blocks per sample (rows 0..511)
W = 1408                  # superplane width

_cache = {}

_SIM = os.environ.get("KERNEL_SIM", "0") == "1"

last_exec_time_ns = None
last_trace = None


def _emit(nc, tc, ctx, bassmod, aps):
    import concourse.bass as bass
    import concourse.mybir as mybir

    f32 = mybir.dt.float32
    bf16 = mybir.dt.bfloat16
    P = 128

    ta1d, ta2d, trd, t1d, jmatd, spd, srcd = aps

    const = ctx.enter_context(tc.tile_pool(name="const", bufs=1))
    gpool = ctx.enter_context(tc.tile_pool(name="gather", bufs=BS))
    mpool = ctx.enter_context(tc.tile_pool(name="mtiles", bufs=2))
    bxpool = ctx.enter_context(tc.tile_pool(name="bx", bufs=2))
    fpool = ctx.enter_context(tc.tile_pool(name="flip", bufs=2))
    pa = ctx.enter_context(tc.tile_pool(name="pa", bufs=2, space="PSUM"))
    pj = ctx.enter_context(tc.tile_pool(name="pj", bufs=2, space="PSUM"))
    spool = ctx.enter_context(tc.tile_pool(name="small", bufs=1))

    REV32 = list(range(31, -1, -1))   # reverse within each 32-lane quadrant

    # constant tables
    ta1sb = const.tile([6, BS * N], bf16)
    nc.sync.dma_start(ta1sb.rearrange("p (s n) -> p s n", s=BS),
                      ta1d.transpose([1, 0, 2]))
    ta2sb = const.tile([6, BS * N], bf16)
    nc.sync.dma_start(ta2sb.rearrange("p (s n) -> p s n", s=BS),
                      ta2d.transpose([1, 0, 2]))
    trsb = const.tile([6, BS * N], bf16)
    nc.sync.dma_start(trsb.rearrange("p (s n) -> p s n", s=BS),
                      trd.transpose([1, 0, 2]))
    jmat = const.tile([P, P], f32)
    nc.sync.dma_start(jmat[:, :], jmatd[:, :])

    NN = N * N
    src_h = srcd.tensor

    def out_ap(s, row0, nrows):
        # [rows, comps, cols] walk of src[s, :, row0:row0+nrows, :]
        return bassmod.AP(src_h, s * 2 * NN + row0 * N,
                          [[N, nrows], [NN, 2], [1, N]])

    # epilogue inputs load first (tiny) so its DVE/ACT work fills the
    # startup gap while the first big gather is in flight
    yrALL = spool.tile([BS, N], f32)
    nc.sync.dma_start(yrALL[:, :], t1d[:, 0, :])
    yiALL = spool.tile([BS, N], f32)
    nc.sync.dma_start(yiALL[:, :], t1d[:, 1, :])

    # all superplane gathers issued up front so no store blocks them.
    # sample 0 is split per component so the first multiply only waits for
    # the real half (subtile deps let DVE start ~3us earlier).
    SGs = []
    for s in range(BS):
        # fused circulant superplane gather: SG[p, c*W + t] = y2c[639 + t - p]
        SG = gpool.tile([P, 2 * W], f32, tag="SG")
        if s == 0:
            nc.sync.dma_start(SG[:, 0:W], spd[s][:, 0, :])
            nc.sync.dma_start(SG[:, W:2 * W], spd[s][:, 1, :])
        else:
            nc.sync.dma_start(SG.rearrange("p (c t) -> p c t", c=2), spd[s])
        SGs.append(SG)

    # ---- row 0 (self-paired under the flip), batched across samples ----
    # Bx[0, j] = y_0 * conj(y_j) * y_j ; C row for i=0 is just y itself.

    yr0 = yrALL[:, 0:1]
    yi0 = yiALL[:, 0:1]

    tr1 = spool.tile([BS, N], f32, tag="tr1")
    nc.scalar.mul(tr1[:, :], yrALL[:, :], yr0)
    tr2 = spool.tile([BS, N], f32, tag="tr2")
    nc.scalar.mul(tr2[:, :], yiALL[:, :], yi0)
    A0r = spool.tile([BS, N], f32)
    nc.vector.tensor_add(A0r[:, :], tr1[:, :], tr2[:, :])

    ti1 = spool.tile([BS, N], f32, tag="ti1")
    nc.scalar.mul(ti1[:, :], yrALL[:, :], yi0)
    ti2 = spool.tile([BS, N], f32, tag="ti2")
    nc.scalar.mul(ti2[:, :], yiALL[:, :], yr0)
    A0i = spool.tile([BS, N], f32)
    nc.vector.tensor_sub(A0i[:, :], ti1[:, :], ti2[:, :])

    mm1 = spool.tile([BS, N], f32, tag="tr1")
    nc.vector.tensor_mul(mm1[:, :], A0r[:, :], yrALL[:, :])
    mm2 = spool.tile([BS, N], f32, tag="tr2")
    nc.vector.tensor_mul(mm2[:, :], A0i[:, :], yiALL[:, :])
    mm3 = spool.tile([BS, N], f32, tag="ti1")
    nc.vector.tensor_mul(mm3[:, :], A0r[:, :], yiALL[:, :])
    mm4 = spool.tile([BS, N], f32, tag="ti2")
    nc.vector.tensor_mul(mm4[:, :], A0i[:, :], yrALL[:, :])

    R0 = spool.tile([BS, 2 * N], f32)
    nc.gpsimd.tensor_sub(R0[:, 0:N], mm1[:, :], mm2[:, :])
    nc.gpsimd.tensor_add(R0[:, N:2 * N], mm3[:, :], mm4[:, :])

    r0_ap = bassmod.AP(src_h, 0, [[2 * NN, BS], [NN, 2], [1, N]])
    nc.sync.dma_start(r0_ap, R0.rearrange("p (c n) -> p c n", c=2))

    for s in range(BS):
        SG = SGs[s]
        for p0 in range(NBLK):
            # block rows: 1 + 128*p0 .. 128 + 128*p0  (rows 1..512)
            r_base = 1 + 128 * p0
            Ar = pa.tile([P, N], f32, tag="Ar")
            Ai = pa.tile([P, N], f32, tag="Ai", bufs=1)
            lhs1 = ta1sb[:, s * N + r_base: s * N + r_base + 128]
            lhs2 = ta2sb[:, s * N + r_base: s * N + r_base + 128]
            for h in range(2):
                rhs = trsb[:, s * N + 512 * h: s * N + 512 * h + 512]
                nc.tensor.matmul(Ar[:, 512 * h: 512 * h + 512], lhs1, rhs,
                                 start=True, stop=True)
                nc.tensor.matmul(Ai[:, 512 * h: 512 * h + 512], lhs2, rhs,
                                 start=True, stop=True)

            c0 = 384 - 128 * p0
            Crs = SG[:, c0:c0 + N]
            Cis = SG[:, W + c0:W + c0 + N]

            # order: both Ar readers first, then Ai readers (frees Ar early)
            m1 = mpool.tile([P, N], f32, tag="m1")
            nc.vector.tensor_mul(m1[:, :], Ar[:, :], Crs)
            m3 = mpool.tile([P, N], f32, tag="m3")
            nc.vector.tensor_mul(m3[:, :], Ar[:, :], Cis)
            m2 = mpool.tile([P, N], f32, tag="m2")
            nc.vector.tensor_mul(m2[:, :], Ai[:, :], Cis)
            m4 = mpool.tile([P, N], f32, tag="m4")
            nc.vector.tensor_mul(m4[:, :], Ai[:, :], Crs)

            # fused [real | imag] plane
            Bx = bxpool.tile([P, 2 * N], f32, tag="Bx")
            nc.gpsimd.tensor_sub(Bx[:, 0:N], m1[:, :], m2[:, :])
            nc.gpsimd.tensor_add(Bx[:, N:2 * N], m3[:, :], m4[:, :])

            # direct store
            nc.sync.dma_start(out_ap(s, r_base, 128),
                              Bx.rearrange("p (c n) -> p c n", c=2))

            # conjugate flip, split across engines:
            #  - real plane: full row reversal on PE (anti-identity, exact
            #    fp32), ACT evacuates PSUM with reversed-column reads
            #  - imag plane: ACT negates, DVE quadrant-reverse shuffle with
            #    reversed-column read; quadrant swap folds into the stores
            F = fpool.tile([P, 2 * N], f32, tag="F")
            RBh0 = pj.tile([P, 512], f32, tag="RBh")
            nc.tensor.matmul(RBh0[:, :], jmat[:, :], Bx[:, 0:512],
                             start=True, stop=True)
            RBh1 = pj.tile([P, 512], f32, tag="RBh")
            nc.tensor.matmul(RBh1[:, :], jmat[:, :], Bx[:, 512:N],
                             start=True, stop=True)
            nc.scalar.copy(F[:, 1:512], RBh1[:, 511:0:-1])
            nc.scalar.copy(F[:, 512:513], RBh1[:, 0:1])
            nc.scalar.copy(F[:, 513:N], RBh0[:, 511:0:-1])
            nc.scalar.copy(F[:, 0:1], RBh0[:, 0:1])

            Tn = fpool.tile([P, N + 1], f32, tag="Tn")
            nc.scalar.mul(Tn[:, 0:N], Bx[:, N:2 * N], -1.0)
            nc.scalar.mul(Tn[:, N:N + 1], Bx[:, N:N + 1], -1.0)
            nc.vector.stream_shuffle(F[:, N:2 * N], Tn[:, N:0:-1], REV32)

            # derived stores (imag was only quadrant-reversed on DVE; real
            # was fully row-reversed on PE -> different partition mappings)
            # real: F[p] -> row 896 - 128*p0 + p ; imag: F[32q+u] -> row
            # (992 - 128*p0 - 32q) + u. Store the two comps separately.
            nc.sync.dma_start(
                bassmod.AP(src_h, s * 2 * NN + (896 - 128 * p0) * N,
                           [[N, 128], [1, N]]),
                F[:, 0:N])
            for q in range(4):
                nc.sync.dma_start(
                    bassmod.AP(src_h,
                               s * 2 * NN + NN + (992 - 128 * p0 - 32 * q) * N,
                               [[N, 32], [1, N]]),
                    F[32 * q:32 * q + 32, N:2 * N])

def _build():
    if "nc" in _cache:
        return _cache["nc"]
    from contextlib import ExitStack

    import concourse.bass as bass
    import concourse.bacc as bacc
    import concourse.mybir as mybir
    import concourse.tile as tile

    f32 = mybir.dt.float32
    bf16 = mybir.dt.bfloat16
    nc = bacc.Bacc("TRN2", target_bir_lowering=False, debug=False,
                   num_devices=NCORES)
    ta1d = nc.dram_tensor("ta1", [BS, 6, N], bf16, kind="ExternalInput").ap()
    ta2d = nc.dram_tensor("ta2", [BS, 6, N], bf16, kind="ExternalInput").ap()
    trd = nc.dram_tensor("tr", [BS, 6, N], bf16, kind="ExternalInput").ap()
    t1d = nc.dram_tensor("t1", [BS, 2, N], f32, kind="ExternalInput").ap()
    jmatd = nc.dram_tensor("jmat", [128, 128], f32, kind="ExternalInput").ap()
    spd = nc.dram_tensor("sp", [BS, 128, 2, W], f32, kind="ExternalInput").ap()
    srcd = nc.dram_tensor("src", [BS, 2, N, N], f32, kind="ExternalOutput").ap()

    aps = (ta1d, ta2d, trd, t1d, jmatd, spd, srcd)

    with tile.TileContext(nc) as tc:
        with ExitStack() as ctx:
            _emit(nc, tc, ctx, bass, aps)

    nc.compile()

    _cache["nc"] = nc
    return nc


def _host_prep(target):
    import ml_dtypes
    bf16 = ml_dtypes.bfloat16

    x = np.asarray(target)[:, 0, :].astype(np.float64)
    y = np.fft.fft(x)
    yr = np.ascontiguousarray(y.real.astype(np.float32))
    yi = np.ascontiguousarray(y.imag.astype(np.float32))

    yrh = yr.astype(bf16).astype(np.float32)
    yrl = (yr - yrh).astype(bf16).astype(np.float32)
    yih = yi.astype(bf16).astype(np.float32)
    yil = (yi - yih).astype(bf16).astype(np.float32)

    ta1 = np.ascontiguousarray(
        np.stack([yrh, yrh, yrl, yih, yih, yil], axis=1).astype(bf16))
    ta2 = np.ascontiguousarray(
        np.stack([yih, yih, yil, -yrh, -yrh, -yrl], axis=1).astype(bf16))
    tr = np.ascontiguousarray(
        np.stack([yrh, yrl, yrh, yih, yil, yih], axis=1).astype(bf16))
    t1 = np.ascontiguousarray(np.stack([yr, yi], axis=1))

    y2r = np.concatenate([yr, yr], axis=1)
    y2i = np.concatenate([yi, yi], axis=1)

    # superplane SP[s, p, c, t] = y2c[s, 639 + t - p]  (partition-major)
    pidx = np.arange(128)[:, None]
    tidx = np.arange(W)[None, :]
    gidx = 639 + tidx - pidx          # [128, W]
    sp = np.ascontiguousarray(
        np.stack([y2r[:, gidx], y2i[:, gidx]], axis=2))

    jmat = np.zeros((128, 128), np.float32)
    jmat[np.arange(128), 127 - np.arange(128)] = 1.0

    return ta1, ta2, tr, t1, jmat, sp


def kernel(**inputs):
    global last_exec_time_ns, last_trace
    target = np.asarray(inputs["target"], dtype=np.float32)
    ta1, ta2, tr, t1, jmat, sp = _host_prep(target)

    nc = _build()

    in_maps = []
    for c in range(NCORES):
        sl = slice(c * BS, (c + 1) * BS)
        in_maps.append({
            "ta1": ta1[sl], "ta2": ta2[sl], "tr": tr[sl], "t1": t1[sl],
            "jmat": jmat, "sp": sp[sl],
        })

    if _SIM:
        from concourse.bass_interp import CoreSim
        outs = []
        for c in range(NCORES):
            sim = CoreSim(nc, trace=False)
            for k, v in in_maps[c].items():
                sim.tensor(k)[:] = v
            sim.simulate(check_with_hw=False)
            outs.append(np.array(sim.tensor("src")))
        source = np.concatenate(outs, axis=0)
    else:
        from concourse.bass_utils import run_bass_kernel_spmd
        res = None
        for attempt in range(3):
            try:
                res = run_bass_kernel_spmd(nc, in_maps,
                                           core_ids=list(range(NCORES)))
                break
            except Exception:
                if attempt == 2:
                    raise
                import time as _time
                _time.sleep(2.0)
        last_exec_time_ns = res.exec_time_ns
        last_trace = res.instructions_and_trace
        source = np.concatenate([r["src"] for r in res.results], axis=0)

    return source, target
